# revision 11
# baseline (speedup 1.0000x reference)
"""GCN 2-layer kernel on 8 Trainium2 NeuronCores (Bass/Tile) — v2.

Sharding: core m owns dest rows [m*R, (m+1)*R). Edges partitioned by dest
row (core), then dest sub-chunk (SUBROWS rows, multiple of 128), then source
chunk (8 GPSIMD core-groups per NC), sorted by dest row.

Per layer SpMM (dest-sharded, AllGather of support rows):
  - table (128, R_pad) f32 in SBUF: partition 16g+f = feature f of source
    chunk g (= core g's rows), delivered by chunked partition-axis AllGather
  - ap_gather (GPSIMD): per-group edge-ordered gather from table
  - TensorE one-hot matmul broadcasts compact (8, L) edge vals to (128, L)
    PSUM (replaces the 16x-replicated HBM val stream of v1)
  - DVE: multiply, prefix scan; ap_gather #2 extracts prefix at per-row end
    positions; DVE shifted subtract -> per-(row,group) segment sums
  - TensorE one-hot SEL matmul sums the 8 group-partials
Layer 1 emits h in (feat, row) orientation and feeds z2 = W2^T h windows +
chunked AllGather#2 inline. Layer 2 uses the transposed SEL matmul
(dd stationary) to produce row-major tiles directly; log_softmax runs
batched, output leaves via 7 TensorE transposes + one contiguous DMA.

X@W1 runs in bf16 (X cast host-side; fp32 PSUM accumulation).
"""

import sys

for p in ("/opt/trn_rl_repo",):
    if p not in sys.path:
        sys.path.insert(0, p)

import numpy as np
import ml_dtypes

import concourse.bass as bass
import concourse.mybir as mybir
import concourse.tile as tile
from concourse import bacc, library_config

F32 = mybir.dt.float32
F32R = mybir.dt.float32r
BF16 = mybir.dt.bfloat16
I16 = mybir.dt.int16
NPBF = ml_dtypes.bfloat16


class Cfg:
    def __init__(self, N, E, IN, HID, OUT, SUBS, SUBROWS, NW, NCH, BAT):
        self.N = N
        self.E = E
        self.IN = IN
        self.HID = HID          # 16
        self.OUT = OUT          # 7
        self.C = 8
        self.R = N // 8         # real rows per core / source chunk
        self.SUBS = SUBS
        self.SUBROWS = SUBROWS  # rows per sub-chunk; multiple of 128
        self.RP = SUBS * SUBROWS  # padded rows per core
        self.NH = SUBROWS // 128  # 128-row tiles per sub
        self.NW = NW            # phase-A column windows
        self.WCOL = self.RP // NW
        self.NCH = NCH          # allgather chunks
        self.BAT = BAT          # subs per batched gather/scan/extract
        self.CH = self.RP // NCH
        import math
        ealign = 32 // math.gcd(BAT, 32)
        # batch-start idx slices (k0*EXT//16) must be 4B-aligned uint32 reads
        self.EXT = -(-(SUBROWS + 1) // ealign) * ealign
        assert (BAT * self.EXT) % 32 == 0
        self.EXTP = -(-(SUBS * self.EXT) // 32) * 32  # padded eidx stream
        self.KC = -(-IN // 128)
        self.KLAST = IN - (self.KC - 1) * 128
        self.NRT2 = self.RP // 128       # row tiles incl. padding
        self.NRT = -(-self.R // 128)     # row tiles with real rows
        assert SUBROWS % 128 == 0
        assert self.RP >= self.R and self.RP % NW == 0 and self.WCOL <= 512
        assert self.CH % self.WCOL == 0 and self.CH % SUBROWS == 0
        assert self.RP * 4 // 4 <= 2**15  # ap_gather num_elems limit (f32)


FULL = Cfg(N=100_000, E=3_200_000, IN=1433, HID=16, OUT=7,
           SUBS=50, SUBROWS=256, NW=25, NCH=1, BAT=8)


def prepare(x, adj_row, adj_col, adj_val, W1, b1, W2, b2, cfg):
    """Host preprocessing: build per-core input maps (vectorized numpy)."""
    N, E, R, C = cfg.N, cfg.E, cfg.R, cfg.C
    SUBS, SUBROWS, EXT = cfg.SUBS, cfg.SUBROWS, cfg.EXT

    adj_row = np.asarray(adj_row)
    adj_col = np.asarray(adj_col)
    adj_val = np.asarray(adj_val, dtype=np.float32)

    core = adj_row // R
    row_rel = adj_row - core * R
    sub = row_rel // SUBROWS
    grp = adj_col // R
    order = np.lexsort((adj_row, grp, sub, core))
    r_s = adj_row[order]
    c_s = adj_col[order]
    v_s = adj_val[order]
    core_s = core[order]
    grp_s = grp[order]
    sub_s = sub[order]

    key = (core_s * SUBS + sub_s) * C + grp_s
    ncell = C * SUBS * C
    starts = np.searchsorted(key, np.arange(ncell))
    ends = np.searchsorted(key, np.arange(ncell) + 1)
    cnt = (ends - starts).reshape(C, SUBS, C)

    glk = cnt.max(axis=(0, 2)) + 1
    # per-sub streams only need %4 (num_idxs); batch-START offsets offs[q*BAT]
    # must be 32-aligned so the gather ucode's 4B-aligned uint32 idx reads
    # land on even int16 columns — pad the last sub of each batch.
    glk = (-(-glk // 4) * 4).astype(np.int64)
    glk = np.maximum(glk, 32)
    for q in range(-(-SUBS // cfg.BAT)):
        k0, k1 = q * cfg.BAT, min(SUBS, (q + 1) * cfg.BAT)
        rem = int(glk[k0:k1].sum()) % 32
        if rem:
            glk[k1 - 1] += 32 - rem
    gtot = int(glk.sum())
    offs = np.concatenate([[0], np.cumsum(glk)])

    # vectorized stream fill: slot = offs[sub] + 1 + rank-within-cell
    rank = np.arange(E) - starts[key]
    pos = offs[sub_s] + 1 + rank
    gidx_full = np.zeros((C, C, gtot), np.int16)
    valc_full = np.zeros((C, C, gtot), np.float32)
    gidx_full[core_s, grp_s, pos] = (c_s - grp_s * R).astype(np.int16)
    valc_full[core_s, grp_s, pos] = v_s

    # extraction positions: per (core, sub, grp, row) right-searchsorted
    rr = (r_s - core_s * R) - sub_s * SUBROWS
    idx4 = key * SUBROWS + rr
    cnt4 = np.bincount(idx4, minlength=ncell * SUBROWS).reshape(
        C, SUBS, C, SUBROWS)
    pos4 = cnt4.cumsum(axis=3)
    ex = np.zeros((C, SUBS, C, EXT), np.int64)
    ex[..., 1:1 + SUBROWS] = pos4
    ex[..., 1 + SUBROWS:] = pos4[..., -1:]
    # batch-local stream offsets: sub k's extract indices point into the
    # batched gather buffer [offs[k0], offs[k1])
    for k in range(SUBS):
        k0 = (k // cfg.BAT) * cfg.BAT
        ex[:, k] += offs[k] - offs[k0]
    assert ex.max() < 2**15
    ex = ex.astype(np.int16)

    def wrap16(a):
        # (8, L) -> (128, L//16): out[16g+p, s] = a[g, s*16+p]
        Cg, L = a.shape
        return np.ascontiguousarray(
            a.reshape(Cg, L // 16, 16).transpose(0, 2, 1).reshape(
                Cg * 16, L // 16))

    W1f = np.asarray(W1, dtype=np.float32)
    w1p = np.zeros((cfg.KC * 128, cfg.HID), np.float32)
    w1p[: cfg.IN] = W1f
    w1p = w1p.astype(NPBF)

    sel1 = np.zeros((128, 16), np.float32)
    sel1[np.arange(128), np.arange(128) % 16] = 1.0
    sel2 = np.zeros((128, 16), np.float32)
    for p in range(128):
        if p % 16 < cfg.OUT:
            sel2[p, p % 16] = 1.0
    b2r = np.zeros((1, 16), np.float32)
    b2r[0, : cfg.OUT] = np.asarray(b2, dtype=np.float32).reshape(-1)
    sel8 = np.zeros((8, 128), np.float32)
    for g in range(8):
        sel8[g, 16 * g:16 * g + 16] = 1.0
    sel8 = sel8.astype(NPBF)
    ident = np.eye(128, dtype=np.float32)

    x = np.asarray(x)
    in_maps = []
    for m in range(C):
        xt = np.ascontiguousarray(
            x[m * R:(m + 1) * R].T.astype(NPBF))
        e2 = np.zeros((C, cfg.EXTP), np.int16)
        e2[:, : SUBS * EXT] = ex[m].transpose(1, 0, 2).reshape(
            C, SUBS * EXT)
        in_maps.append(
            dict(
                xt=xt,
                w1p=w1p,
                b1=np.ascontiguousarray(
                    np.asarray(b1, np.float32).reshape(cfg.HID, 1)),
                w2=np.ascontiguousarray(np.asarray(W2, np.float32)),
                gidx=wrap16(gidx_full[m]),
                eidx=wrap16(e2),
                valc=np.ascontiguousarray(valc_full[m].astype(NPBF)),
                sel1=sel1,
                sel2=sel2,
                b2r=b2r,
                sel8=sel8,
                ident=ident,
            )
        )
    return in_maps, glk, offs


def build(cfg, glk, offs, dbg=False):
    nc = bacc.Bacc("TRN2", target_bir_lowering=False, debug=False,
                   num_devices=cfg.C)
    R, RP, HID, OUT = cfg.R, cfg.RP, cfg.HID, cfg.OUT
    SUBS, SUBROWS, EXT, NH = cfg.SUBS, cfg.SUBROWS, cfg.EXT, cfg.NH
    NW, WCOL, NCH, CH = cfg.NW, cfg.WCOL, cfg.NCH, cfg.CH
    KC, KLAST = cfg.KC, cfg.KLAST
    NRT2, NRT = cfg.NRT2, cfg.NRT
    gtot = int(glk.sum())
    glkmax = int(max(glk))
    WPC = NW // NCH    # windows per AG1 chunk
    SPC = SUBS // NCH  # subs per AG2 chunk
    # last sub with any real rows
    KSUB = -(-R // SUBROWS)  # number of subs covering real rows
    RLAST = R - (NRT - 1) * 128  # rows in last real 128-tile

    xt = nc.dram_tensor("xt", [cfg.IN, R], BF16, kind="ExternalInput").ap()
    w1p = nc.dram_tensor("w1p", [KC * 128, HID], BF16, kind="ExternalInput").ap()
    b1 = nc.dram_tensor("b1", [HID, 1], F32, kind="ExternalInput").ap()
    w2 = nc.dram_tensor("w2", [HID, OUT], F32, kind="ExternalInput").ap()
    gidx = nc.dram_tensor("gidx", [128, gtot // 16], I16, kind="ExternalInput").ap()
    eidx = nc.dram_tensor("eidx", [128, cfg.EXTP // 16], I16, kind="ExternalInput").ap()
    valc = nc.dram_tensor("valc", [8, gtot], BF16, kind="ExternalInput").ap()
    sel1 = nc.dram_tensor("sel1", [128, 16], F32, kind="ExternalInput").ap()
    sel2 = nc.dram_tensor("sel2", [128, 16], F32, kind="ExternalInput").ap()
    b2r = nc.dram_tensor("b2r", [1, 16], F32, kind="ExternalInput").ap()
    sel8 = nc.dram_tensor("sel8", [8, 128], BF16, kind="ExternalInput").ap()
    ident = nc.dram_tensor("ident", [128, 128], F32, kind="ExternalInput").ap()
    out = nc.dram_tensor("out", [R, OUT], F32, kind="ExternalOutput").ap()
    if dbg:
        dbg_tab = nc.dram_tensor("dbg_tab", [128, RP], F32, kind="ExternalOutput").ap()
        dbg_tab2 = nc.dram_tensor("dbg_tab2", [128, RP], F32, kind="ExternalOutput").ap()
        dbg_h = nc.dram_tensor("dbg_h", [HID, RP], F32, kind="ExternalOutput").ap()
        dbg_zr = nc.dram_tensor("dbg_zr", [128, NRT2 * OUT], F32, kind="ExternalOutput").ap()

    rg = [list(range(cfg.C))]

    with tile.TileContext(nc) as tc:
        BAT = cfg.BAT
        NBAT = -(-SUBS // BAT)
        bmax = 0
        for q in range(NBAT):
            k0, k1 = q * BAT, min(SUBS, (q + 1) * BAT)
            bmax = max(bmax, int(offs[k1] - offs[k0]))
        with (
            tc.tile_pool(name="const", bufs=1) as cpool,
            tc.tile_pool(name="tab", bufs=1) as tabpool,
            tc.tile_pool(name="soft", bufs=1) as sfpool,
            tc.tile_pool(name="hz", bufs=2) as hpool,
            tc.tile_pool(name="stz", bufs=2) as zpool,
            tc.tile_pool(name="st1", bufs=2) as stpool,
            tc.tile_pool(name="psA", bufs=2, space="PSUM") as ppool,
            tc.tile_pool(name="psB", bufs=1, space="PSUM") as ppool2,
            tc.tile_pool(name="dram", bufs=1, space="DRAM") as dpool,
        ):
            nc.gpsimd.load_library(library_config.ap_gather)

            # ---- consts + index prefetch (sync queue)
            w1s = cpool.tile([128, KC, HID], BF16)
            nc.sync.dma_start(w1s[:], w1p.rearrange("(k p) m -> p k m", p=128))
            b1s = cpool.tile([HID, 1], F32)
            nc.sync.dma_start(b1s[:], b1[:])
            w2s = cpool.tile([HID, OUT], F32)
            nc.sync.dma_start(w2s[:], w2[:])
            sel1s = cpool.tile([128, 16], F32)
            nc.sync.dma_start(sel1s[:], sel1[:])
            sel2s = cpool.tile([128, 16], F32)
            nc.sync.dma_start(sel2s[:], sel2[:])
            b2rs = cpool.tile([1, 16], F32)
            nc.sync.dma_start(b2rs[:], b2r[:])
            sel8s = cpool.tile([8, 128], BF16)
            nc.sync.dma_start(sel8s[:], sel8[:])
            idents = cpool.tile([128, 128], F32)
            nc.sync.dma_start(idents[:], ident[:])
            gidx_s = cpool.tile([128, gtot // 16], I16)
            nc.sync.dma_start(gidx_s[:], gidx[:])
            eidx_s = cpool.tile([128, cfg.EXTP // 16], I16)
            nc.sync.dma_start(eidx_s[:], eidx[:])
            ones1 = cpool.tile([128, 1], F32)
            nc.vector.memset(ones1[:], 1.0)
            onesr = cpool.tile([1, 128], F32)
            nc.vector.memset(onesr[:], 1.0)

            def ones_b(L):
                # stride-0 broadcast of the per-partition 1.0 along free dim
                return bass.AP(ones1.tensor, ones1.offset,
                               [ones1.ap[0], [0, L]])

            # ---- DRAM collective chunk tiles
            agin1 = [dpool.tile([HID, CH], F32, name=f"agin1_{c}")
                     for c in range(NCH)]
            agout1 = [dpool.tile([128, CH], F32, addr_space="Shared",
                                 name=f"agout1_{c}") for c in range(NCH)]
            agin2 = [dpool.tile([16, CH], F32, name=f"agin2_{c}")
                     for c in range(NCH)]
            agout2 = [dpool.tile([128, CH], F32, addr_space="Shared",
                                 name=f"agout2_{c}") for c in range(NCH)]

            # ---- phase A: (X @ W1)^T windows -> agin1 chunks, chunked AG1
            # (xw pool scoped so its SBUF returns before the stream buffers)
            with tc.tile_pool(name="xw", bufs=2) as xpool:
                for w in range(NW):
                    c0 = w * WCOL
                    cols = min(WCOL, R - c0) if c0 < R else 0
                    ch = w // WPC
                    if cols > 0:
                        xw = xpool.tile([128, KC, WCOL], BF16, tag="xw")
                        if KC > 1:
                            nc.sync.dma_start(
                                xw[:, : KC - 1, :cols],
                                xt[: (KC - 1) * 128, c0:c0 + cols].rearrange(
                                    "(k p) c -> p k c", p=128),
                            )
                        nc.sync.dma_start(
                            xw[:KLAST, KC - 1, :cols],
                            xt[(KC - 1) * 128:, c0:c0 + cols],
                        )
                        pa = ppool.tile([HID, WCOL], F32, tag="pa")
                        for k in range(KC):
                            kp = 128 if k < KC - 1 else KLAST
                            nc.tensor.matmul(
                                pa[:, :cols], w1s[:kp, k, :], xw[:kp, k, :cols],
                                start=(k == 0), stop=(k == KC - 1),
                            )
                        st1 = stpool.tile([HID, WCOL], F32, tag="st1")
                        nc.scalar.copy(st1[:, :cols], pa[:, :cols])
                        nc.scalar.dma_start(
                            agin1[ch][:, (w % WPC) * WCOL:(w % WPC) * WCOL + cols],
                            st1[:, :cols],
                        )
                    if w % WPC == WPC - 1:
                        nc.gpsimd.collective_compute(
                            "AllGather", mybir.AluOpType.bypass,
                            ins=[agin1[ch].opt()], outs=[agout1[ch].opt()],
                            replica_groups=rg,
                        )

            # ---- table1 loads (sync queue, after phase A)
            table = tabpool.tile([128, RP], F32, tag="table")
            for c in range(NCH):
                nc.sync.dma_start(table[:, c * CH:(c + 1) * CH], agout1[c][:])
            if dbg:
                nc.sync.dma_start(dbg_tab[:], table[:])

            with (
                tc.tile_pool(name="stream", bufs=2) as spool,
                tc.tile_pool(name="vals", bufs=1) as vpool,
                tc.tile_pool(name="rx", bufs=1) as rxpool,
                tc.tile_pool(name="ddp", bufs=2) as ddpool,
            ):
                def batch_gather(tbl, q):
                    k0, k1 = q * BAT, min(SUBS, (q + 1) * BAT)
                    o0, o1 = int(offs[k0]), int(offs[k1])
                    Lb = o1 - o0
                    vals = vpool.tile([8, bmax], BF16, tag="vals")
                    nc.sync.dma_start(vals[:, :Lb], valc[:, o0:o1])
                    buf = spool.tile([128, bmax], F32, tag="buf")
                    nc.gpsimd.ap_gather(
                        buf[:, :Lb].rearrange("c (n d) -> c n d", d=1),
                        tbl[:].rearrange("c (n d) -> c n d", d=1),
                        gidx_s[:, o0 // 16:o1 // 16],
                        channels=128, num_elems=RP, d=1, num_idxs=Lb)
                    return buf, vals, Lb

                def batch_mult_scan(buf, vals, Lb):
                    for j0 in range(0, Lb, 512):
                        cj = min(512, Lb - j0)
                        vr = ppool.tile([128, 512], F32, tag="vr")
                        nc.tensor.matmul(vr[:, :cj], sel8s[:],
                                         vals[:, j0:j0 + cj],
                                         start=True, stop=True)
                        nc.vector.tensor_mul(buf[:, j0:j0 + cj],
                                             buf[:, j0:j0 + cj], vr[:, :cj])
                    nc.vector.tensor_tensor_scan(
                        buf[:, :Lb], ones_b(Lb), buf[:, :Lb], 0.0,
                        mybir.AluOpType.mult, mybir.AluOpType.add)

                def batch_extract(buf, Lb, q):
                    k0, k1 = q * BAT, min(SUBS, (q + 1) * BAT)
                    BE = (k1 - k0) * EXT
                    rxt = rxpool.tile([128, BAT * EXT], F32, tag="rxt")
                    nc.gpsimd.ap_gather(
                        rxt[:, :BE].rearrange("c (n d) -> c n d", d=1),
                        buf[:, :Lb].rearrange("c (n d) -> c n d", d=1),
                        eidx_s[:, k0 * EXT // 16:-(-(k1 * EXT) // 16)],
                        channels=128, num_elems=Lb, d=1, num_idxs=BE)
                    dd = ddpool.tile([128, BAT * EXT], F32, tag="dd")
                    nc.vector.tensor_sub(dd[:, 1:BE], rxt[:, 1:BE],
                                         rxt[:, :BE - 1])
                    return dd

                def ag2_trigger(c):
                    nc.gpsimd.collective_compute(
                        "AllGather", mybir.AluOpType.bypass,
                        ins=[agin2[c].opt()], outs=[agout2[c].opt()],
                        replica_groups=rg,
                    )

                # ---- layer 1: batched spmm -> h -> z2 -> agin2 (chunked AG2)
                cur = batch_gather(table, 0)
                trig = 0
                for q in range(NBAT):
                    buf, vals, Lb = cur
                    batch_mult_scan(buf, vals, Lb)
                    if q + 1 < NBAT:
                        cur = batch_gather(table, q + 1)
                    dd = batch_extract(buf, Lb, q)
                    k0, k1 = q * BAT, min(SUBS, (q + 1) * BAT)
                    for k in range(k0, min(k1, KSUB)):
                        b = k - k0
                        pb = ppool2.tile([16, SUBROWS], F32, tag="pb")
                        nc.tensor.matmul(
                            pb[:HID], sel1s[:],
                            dd[:, b * EXT + 1:b * EXT + 1 + SUBROWS],
                            start=True, stop=True,
                        )
                        hst = hpool.tile([HID, SUBROWS], F32, tag="hst")
                        nc.scalar.activation(
                            hst[:], pb[:HID],
                            mybir.ActivationFunctionType.Relu, bias=b1s[:])
                        if dbg:
                            nc.sync.dma_start(
                                dbg_h[:, k * SUBROWS:(k + 1) * SUBROWS], hst[:])
                        pz = ppool2.tile([OUT, SUBROWS], F32, tag="pz")
                        nc.tensor.matmul(pz[:], w2s[:], hst[:],
                                         start=True, stop=True)
                        stz = zpool.tile([16, SUBROWS], F32, tag="stz")
                        if k < 2:
                            nc.vector.memset(stz[:], 0.0)
                        nc.scalar.copy(stz[:OUT], pz[:])
                        c2 = k // SPC
                        nc.scalar.dma_start(
                            agin2[c2][:, (k % SPC) * SUBROWS:
                                      (k % SPC + 1) * SUBROWS],
                            stz[:],
                        )
                    while trig < NCH - 1 and SPC * (trig + 1) + 1 < k1:
                        ag2_trigger(trig)
                        trig += 1
                for c in range(trig, NCH):
                    ag2_trigger(c)

                # ---- table2 loads
                table2 = tabpool.tile([128, RP], F32, tag="table")
                for c in range(NCH):
                    nc.sync.dma_start(
                        table2[:, c * CH:(c + 1) * CH], agout2[c][:])
                if dbg:
                    nc.sync.dma_start(dbg_tab2[:], table2[:])

                # ---- layer 2: batched spmm -> transposed SEL -> zr row tiles
                zr = sfpool.tile([128, NRT2, OUT], F32, tag="zr")
                cur = batch_gather(table2, 0)
                for q in range(NBAT):
                    buf, vals, Lb = cur
                    batch_mult_scan(buf, vals, Lb)
                    if q + 1 < NBAT:
                        cur = batch_gather(table2, q + 1)
                    dd = batch_extract(buf, Lb, q)
                    k0, k1 = q * BAT, min(SUBS, (q + 1) * BAT)
                    for k in range(k0, min(k1, KSUB)):
                        b = k - k0
                        for h in range(NH):
                            t = k * NH + h
                            if t >= NRT:
                                break
                            pbT = ppool2.tile([128, 16], F32, tag="pbT")
                            nc.tensor.matmul(
                                pbT[:],
                                dd[:, b * EXT + 1 + 128 * h:
                                   b * EXT + 1 + 128 * h + 128],
                                sel2s[:],
                                start=True, stop=False,
                            )
                            nc.tensor.matmul(
                                pbT[:], onesr[:], b2rs[:],
                                start=False, stop=True,
                            )
                            nc.scalar.copy(zr[:, t, :], pbT[:, :OUT])

            if dbg:
                nc.sync.dma_start(
                    dbg_zr[:], zr[:].rearrange("p t j -> p (t j)"))
            # ---- log_softmax over OUT (batched)
            mx = sfpool.tile([128, NRT], F32, tag="mx")
            nc.vector.tensor_reduce(
                mx[:], zr[:, :NRT, :], axis=mybir.AxisListType.X,
                op=mybir.AluOpType.max)
            zs = sfpool.tile([128, OUT, NRT], F32, tag="zs")
            for j in range(OUT):
                nc.vector.tensor_sub(zs[:, j, :], zr[:, :NRT, j], mx[:])
            exb = sfpool.tile([128, OUT, NRT], F32, tag="exb")
            nc.scalar.activation(exb[:], zs[:], mybir.ActivationFunctionType.Exp)
            sm = sfpool.tile([128, NRT], F32, tag="sm")
            nc.vector.tensor_add(sm[:], exb[:, 0, :], exb[:, 1, :])
            for j in range(2, OUT):
                nc.vector.tensor_add(sm[:], sm[:], exb[:, j, :])
            lg = sfpool.tile([128, NRT], F32, tag="lg")
            nc.scalar.activation(lg[:], sm[:], mybir.ActivationFunctionType.Ln)
            for j in range(OUT):
                nc.vector.tensor_sub(zs[:, j, :], zs[:, j, :], lg[:])

            # ---- output: 7 transposes -> zo (NRT, 128, OUT) -> contiguous DMA
            zo = sfpool.tile([NRT, 128, OUT], F32, tag="zo")
            for j in range(OUT):
                pt = ppool2.tile([NRT, 128], F32, tag="pt")
                nc.tensor.matmul(
                    pt[:], zs[:, j, :], idents[:],
                    is_transpose=True, start=True, stop=True,
                )
                nc.scalar.copy(zo[:, :, j], pt[:])
            if NRT > 1:
                nc.sync.dma_start(
                    out[: (NRT - 1) * 128, :].rearrange(
                        "(t p) j -> t (p j)", p=128),
                    zo[: NRT - 1, :, :].rearrange("t p j -> t (p j)"),
                )
            nc.sync.dma_start(
                out[(NRT - 1) * 128:, :].rearrange("(t p) j -> t p j", t=1),
                zo[NRT - 1:NRT, :RLAST, :])

    nc.compile()
    return nc


def kernel(x, adj_row, adj_col, adj_val, W1, b1, W2, b2):
    from concourse import bass_utils

    cfg = FULL
    in_maps, glk, offs = prepare(
        np.asarray(x), np.asarray(adj_row), np.asarray(adj_col),
        np.asarray(adj_val), np.asarray(W1), np.asarray(b1),
        np.asarray(W2), np.asarray(b2), cfg,
    )
    nc = build(cfg, glk, offs)
    res = bass_utils.run_bass_kernel_spmd(nc, in_maps, core_ids=list(range(cfg.C)))
    outs = [res.results[m]["out"] for m in range(cfg.C)]
    return np.concatenate(outs, axis=0)[: cfg.N]



# revision 12
# speedup vs baseline: 1.1636x; 1.1636x over previous
"""GCN 2-layer kernel on 8 Trainium2 NeuronCores (Bass/Tile) — v2.

Sharding: core m owns dest rows [m*R, (m+1)*R). Edges partitioned by dest
row (core), then dest sub-chunk (SUBROWS rows, multiple of 128), then source
chunk (8 GPSIMD core-groups per NC), sorted by dest row.

Per layer SpMM (dest-sharded, AllGather of support rows):
  - table (128, R_pad) f32 in SBUF: partition 16g+f = feature f of source
    chunk g (= core g's rows), delivered by chunked partition-axis AllGather
  - ap_gather (GPSIMD): per-group edge-ordered gather from table
  - TensorE one-hot matmul broadcasts compact (8, L) edge vals to (128, L)
    PSUM (replaces the 16x-replicated HBM val stream of v1)
  - DVE: multiply, prefix scan; ap_gather #2 extracts prefix at per-row end
    positions; DVE shifted subtract -> per-(row,group) segment sums
  - TensorE one-hot SEL matmul sums the 8 group-partials
Layer 1 emits h in (feat, row) orientation and feeds z2 = W2^T h windows +
chunked AllGather#2 inline. Layer 2 uses the transposed SEL matmul
(dd stationary) to produce row-major tiles directly; log_softmax runs
batched, output leaves via 7 TensorE transposes + one contiguous DMA.

X@W1 runs in bf16 (X cast host-side; fp32 PSUM accumulation).
"""

import sys

for p in ("/opt/trn_rl_repo",):
    if p not in sys.path:
        sys.path.insert(0, p)

import numpy as np
import ml_dtypes

import concourse.bass as bass
import concourse.mybir as mybir
import concourse.tile as tile
from concourse import bacc, library_config

F32 = mybir.dt.float32
F32R = mybir.dt.float32r
BF16 = mybir.dt.bfloat16
I16 = mybir.dt.int16
NPBF = ml_dtypes.bfloat16


class Cfg:
    def __init__(self, N, E, IN, HID, OUT, SUBS, SUBROWS, NW, NCH, BAT):
        self.N = N
        self.E = E
        self.IN = IN
        self.HID = HID          # 16
        self.OUT = OUT          # 7
        self.C = 8
        self.R = N // 8         # real rows per core / source chunk
        self.SUBS = SUBS
        self.SUBROWS = SUBROWS  # rows per sub-chunk; multiple of 128
        self.RP = SUBS * SUBROWS  # padded rows per core
        self.NH = SUBROWS // 128  # 128-row tiles per sub
        self.NW = NW            # phase-A column windows
        self.WCOL = self.RP // NW
        self.NCH = NCH          # allgather chunks
        self.BAT = BAT          # subs per batched gather/scan/extract
        self.CH = self.RP // NCH
        self.EXT = -(-(SUBROWS + 1) // 32) * 32  # //16 even: 4B-aligned idx slices
        self.KC = -(-IN // 128)
        self.KLAST = IN - (self.KC - 1) * 128
        self.NRT2 = self.RP // 128       # row tiles incl. padding
        self.NRT = -(-self.R // 128)     # row tiles with real rows
        assert SUBROWS % 128 == 0
        assert self.RP >= self.R and self.RP % NW == 0 and self.WCOL <= 512
        assert self.CH % self.WCOL == 0 and self.CH % SUBROWS == 0
        assert self.RP * 4 // 4 <= 2**15  # ap_gather num_elems limit (f32)


FULL = Cfg(N=100_000, E=3_200_000, IN=1433, HID=16, OUT=7,
           SUBS=50, SUBROWS=256, NW=25, NCH=1, BAT=8)


def prepare(x, adj_row, adj_col, adj_val, W1, b1, W2, b2, cfg):
    """Host preprocessing: build per-core input maps (vectorized numpy)."""
    N, E, R, C = cfg.N, cfg.E, cfg.R, cfg.C
    SUBS, SUBROWS, EXT = cfg.SUBS, cfg.SUBROWS, cfg.EXT

    adj_row = np.asarray(adj_row)
    adj_col = np.asarray(adj_col)
    adj_val = np.asarray(adj_val, dtype=np.float32)

    core = adj_row // R
    row_rel = adj_row - core * R
    sub = row_rel // SUBROWS
    grp = adj_col // R
    order = np.lexsort((adj_row, grp, sub, core))
    r_s = adj_row[order]
    c_s = adj_col[order]
    v_s = adj_val[order]
    core_s = core[order]
    grp_s = grp[order]
    sub_s = sub[order]

    key = (core_s * SUBS + sub_s) * C + grp_s
    ncell = C * SUBS * C
    starts = np.searchsorted(key, np.arange(ncell))
    ends = np.searchsorted(key, np.arange(ncell) + 1)
    cnt = (ends - starts).reshape(C, SUBS, C)

    glk = cnt.max(axis=(0, 2)) + 1
    # round to 32 so offs//16 stays even (4B-aligned int16 idx slices for
    # the gather ucode, which reads the index stream as uint32)
    glk = (-(-glk // 32) * 32).astype(np.int64)
    glk = np.maximum(glk, 32)
    gtot = int(glk.sum())
    offs = np.concatenate([[0], np.cumsum(glk)])

    # vectorized stream fill: slot = offs[sub] + 1 + rank-within-cell
    rank = np.arange(E) - starts[key]
    pos = offs[sub_s] + 1 + rank
    gidx_full = np.zeros((C, C, gtot), np.int16)
    valc_full = np.zeros((C, C, gtot), np.float32)
    gidx_full[core_s, grp_s, pos] = (c_s - grp_s * R).astype(np.int16)
    valc_full[core_s, grp_s, pos] = v_s

    # extraction positions: per (core, sub, grp, row) right-searchsorted
    rr = (r_s - core_s * R) - sub_s * SUBROWS
    idx4 = key * SUBROWS + rr
    cnt4 = np.bincount(idx4, minlength=ncell * SUBROWS).reshape(
        C, SUBS, C, SUBROWS)
    pos4 = cnt4.cumsum(axis=3)
    ex = np.zeros((C, SUBS, C, EXT), np.int64)
    ex[..., 1:1 + SUBROWS] = pos4
    ex[..., 1 + SUBROWS:] = pos4[..., -1:]
    # batch-local stream offsets: sub k's extract indices point into the
    # batched gather buffer [offs[k0], offs[k1])
    for k in range(SUBS):
        k0 = (k // cfg.BAT) * cfg.BAT
        ex[:, k] += offs[k] - offs[k0]
    assert ex.max() < 2**15
    ex = ex.astype(np.int16)

    def wrap16(a):
        # (8, L) -> (128, L//16): out[16g+p, s] = a[g, s*16+p]
        Cg, L = a.shape
        return np.ascontiguousarray(
            a.reshape(Cg, L // 16, 16).transpose(0, 2, 1).reshape(
                Cg * 16, L // 16))

    W1f = np.asarray(W1, dtype=np.float32)
    w1p = np.zeros((cfg.KC * 128, cfg.HID), np.float32)
    w1p[: cfg.IN] = W1f
    w1p = w1p.astype(NPBF)

    sel1 = np.zeros((128, 16), np.float32)
    sel1[np.arange(128), np.arange(128) % 16] = 1.0
    sel2 = np.zeros((128, 16), np.float32)
    for p in range(128):
        if p % 16 < cfg.OUT:
            sel2[p, p % 16] = 1.0
    b2r = np.zeros((1, 16), np.float32)
    b2r[0, : cfg.OUT] = np.asarray(b2, dtype=np.float32).reshape(-1)
    sel8 = np.zeros((8, 128), np.float32)
    for g in range(8):
        sel8[g, 16 * g:16 * g + 16] = 1.0
    sel8 = sel8.astype(NPBF)
    ident = np.eye(128, dtype=np.float32)

    x = np.asarray(x)
    in_maps = []
    for m in range(C):
        xt = np.ascontiguousarray(
            x[m * R:(m + 1) * R].T.astype(NPBF))
        e2 = np.ascontiguousarray(
            ex[m].transpose(1, 0, 2).reshape(C, SUBS * EXT))
        in_maps.append(
            dict(
                xt=xt,
                w1p=w1p,
                b1=np.ascontiguousarray(
                    np.asarray(b1, np.float32).reshape(cfg.HID, 1)),
                w2=np.ascontiguousarray(np.asarray(W2, np.float32)),
                gidx=wrap16(gidx_full[m]),
                eidx=wrap16(e2),
                valc=np.ascontiguousarray(valc_full[m].astype(NPBF)),
                sel1=sel1,
                sel2=sel2,
                b2r=b2r,
                sel8=sel8,
                ident=ident,
            )
        )
    return in_maps, glk, offs


def build(cfg, glk, offs, dbg=False):
    nc = bacc.Bacc("TRN2", target_bir_lowering=False, debug=False,
                   num_devices=cfg.C)
    R, RP, HID, OUT = cfg.R, cfg.RP, cfg.HID, cfg.OUT
    SUBS, SUBROWS, EXT, NH = cfg.SUBS, cfg.SUBROWS, cfg.EXT, cfg.NH
    NW, WCOL, NCH, CH = cfg.NW, cfg.WCOL, cfg.NCH, cfg.CH
    KC, KLAST = cfg.KC, cfg.KLAST
    NRT2, NRT = cfg.NRT2, cfg.NRT
    gtot = int(glk.sum())
    glkmax = int(max(glk))
    WPC = NW // NCH    # windows per AG1 chunk
    SPC = SUBS // NCH  # subs per AG2 chunk
    # last sub with any real rows
    KSUB = -(-R // SUBROWS)  # number of subs covering real rows
    RLAST = R - (NRT - 1) * 128  # rows in last real 128-tile

    xt = nc.dram_tensor("xt", [cfg.IN, R], BF16, kind="ExternalInput").ap()
    w1p = nc.dram_tensor("w1p", [KC * 128, HID], BF16, kind="ExternalInput").ap()
    b1 = nc.dram_tensor("b1", [HID, 1], F32, kind="ExternalInput").ap()
    w2 = nc.dram_tensor("w2", [HID, OUT], F32, kind="ExternalInput").ap()
    gidx = nc.dram_tensor("gidx", [128, gtot // 16], I16, kind="ExternalInput").ap()
    eidx = nc.dram_tensor("eidx", [128, SUBS * EXT // 16], I16, kind="ExternalInput").ap()
    valc = nc.dram_tensor("valc", [8, gtot], BF16, kind="ExternalInput").ap()
    sel1 = nc.dram_tensor("sel1", [128, 16], F32, kind="ExternalInput").ap()
    sel2 = nc.dram_tensor("sel2", [128, 16], F32, kind="ExternalInput").ap()
    b2r = nc.dram_tensor("b2r", [1, 16], F32, kind="ExternalInput").ap()
    sel8 = nc.dram_tensor("sel8", [8, 128], BF16, kind="ExternalInput").ap()
    ident = nc.dram_tensor("ident", [128, 128], F32, kind="ExternalInput").ap()
    out = nc.dram_tensor("out", [R, OUT], F32, kind="ExternalOutput").ap()
    if dbg:
        dbg_tab = nc.dram_tensor("dbg_tab", [128, RP], F32, kind="ExternalOutput").ap()
        dbg_tab2 = nc.dram_tensor("dbg_tab2", [128, RP], F32, kind="ExternalOutput").ap()
        dbg_h = nc.dram_tensor("dbg_h", [HID, RP], F32, kind="ExternalOutput").ap()
        dbg_zr = nc.dram_tensor("dbg_zr", [128, NRT2 * OUT], F32, kind="ExternalOutput").ap()

    rg = [list(range(cfg.C))]

    with tile.TileContext(nc) as tc:
        BAT = cfg.BAT
        NBAT = -(-SUBS // BAT)
        bmax = 0
        for q in range(NBAT):
            k0, k1 = q * BAT, min(SUBS, (q + 1) * BAT)
            bmax = max(bmax, int(offs[k1] - offs[k0]))
        with (
            tc.tile_pool(name="const", bufs=1) as cpool,
            tc.tile_pool(name="tab", bufs=1) as tabpool,
            tc.tile_pool(name="soft", bufs=1) as sfpool,
            tc.tile_pool(name="hz", bufs=2) as hpool,
            tc.tile_pool(name="stz", bufs=2) as zpool,
            tc.tile_pool(name="st1", bufs=2) as stpool,
            tc.tile_pool(name="psA", bufs=2, space="PSUM") as ppool,
            tc.tile_pool(name="psB", bufs=1, space="PSUM") as ppool2,
            tc.tile_pool(name="dram", bufs=1, space="DRAM") as dpool,
        ):
            nc.gpsimd.load_library(library_config.ap_gather)

            # ---- consts + index prefetch (sync queue)
            w1s = cpool.tile([128, KC, HID], BF16)
            nc.sync.dma_start(w1s[:], w1p.rearrange("(k p) m -> p k m", p=128))
            b1s = cpool.tile([HID, 1], F32)
            nc.sync.dma_start(b1s[:], b1[:])
            w2s = cpool.tile([HID, OUT], F32)
            nc.sync.dma_start(w2s[:], w2[:])
            sel1s = cpool.tile([128, 16], F32)
            nc.sync.dma_start(sel1s[:], sel1[:])
            sel2s = cpool.tile([128, 16], F32)
            nc.sync.dma_start(sel2s[:], sel2[:])
            b2rs = cpool.tile([1, 16], F32)
            nc.sync.dma_start(b2rs[:], b2r[:])
            sel8s = cpool.tile([8, 128], BF16)
            nc.sync.dma_start(sel8s[:], sel8[:])
            idents = cpool.tile([128, 128], F32)
            nc.sync.dma_start(idents[:], ident[:])
            gidx_s = cpool.tile([128, gtot // 16], I16)
            nc.sync.dma_start(gidx_s[:], gidx[:])
            eidx_s = cpool.tile([128, SUBS * EXT // 16], I16)
            nc.sync.dma_start(eidx_s[:], eidx[:])
            ones1 = cpool.tile([128, 1], F32)
            nc.vector.memset(ones1[:], 1.0)
            onesr = cpool.tile([1, 128], F32)
            nc.vector.memset(onesr[:], 1.0)

            def ones_b(L):
                # stride-0 broadcast of the per-partition 1.0 along free dim
                return bass.AP(ones1.tensor, ones1.offset,
                               [ones1.ap[0], [0, L]])

            # ---- DRAM collective chunk tiles
            agin1 = [dpool.tile([HID, CH], F32, name=f"agin1_{c}")
                     for c in range(NCH)]
            agout1 = [dpool.tile([128, CH], F32, addr_space="Shared",
                                 name=f"agout1_{c}") for c in range(NCH)]
            agin2 = [dpool.tile([16, CH], F32, name=f"agin2_{c}")
                     for c in range(NCH)]
            agout2 = [dpool.tile([128, CH], F32, addr_space="Shared",
                                 name=f"agout2_{c}") for c in range(NCH)]

            # ---- phase A: (X @ W1)^T windows -> agin1 chunks, chunked AG1
            # (xw pool scoped so its SBUF returns before the stream buffers)
            with tc.tile_pool(name="xw", bufs=2) as xpool:
                for w in range(NW):
                    c0 = w * WCOL
                    cols = min(WCOL, R - c0) if c0 < R else 0
                    ch = w // WPC
                    if cols > 0:
                        xw = xpool.tile([128, KC, WCOL], BF16, tag="xw")
                        if KC > 1:
                            nc.sync.dma_start(
                                xw[:, : KC - 1, :cols],
                                xt[: (KC - 1) * 128, c0:c0 + cols].rearrange(
                                    "(k p) c -> p k c", p=128),
                            )
                        nc.sync.dma_start(
                            xw[:KLAST, KC - 1, :cols],
                            xt[(KC - 1) * 128:, c0:c0 + cols],
                        )
                        pa = ppool.tile([HID, WCOL], F32, tag="pa")
                        for k in range(KC):
                            kp = 128 if k < KC - 1 else KLAST
                            nc.tensor.matmul(
                                pa[:, :cols], w1s[:kp, k, :], xw[:kp, k, :cols],
                                start=(k == 0), stop=(k == KC - 1),
                            )
                        st1 = stpool.tile([HID, WCOL], F32, tag="st1")
                        nc.scalar.copy(st1[:, :cols], pa[:, :cols])
                        nc.scalar.dma_start(
                            agin1[ch][:, (w % WPC) * WCOL:(w % WPC) * WCOL + cols],
                            st1[:, :cols],
                        )
                    if w % WPC == WPC - 1:
                        nc.gpsimd.collective_compute(
                            "AllGather", mybir.AluOpType.bypass,
                            ins=[agin1[ch].opt()], outs=[agout1[ch].opt()],
                            replica_groups=rg,
                        )

            # ---- table1 loads (sync queue, after phase A)
            table = tabpool.tile([128, RP], F32, tag="table")
            for c in range(NCH):
                nc.sync.dma_start(table[:, c * CH:(c + 1) * CH], agout1[c][:])
            if dbg:
                nc.sync.dma_start(dbg_tab[:], table[:])

            with (
                tc.tile_pool(name="stream", bufs=2) as spool,
                tc.tile_pool(name="vals", bufs=1) as vpool,
                tc.tile_pool(name="rx", bufs=1) as rxpool,
                tc.tile_pool(name="ddp", bufs=2) as ddpool,
            ):
                def batch_gather(tbl, q):
                    k0, k1 = q * BAT, min(SUBS, (q + 1) * BAT)
                    o0, o1 = int(offs[k0]), int(offs[k1])
                    Lb = o1 - o0
                    vals = vpool.tile([8, bmax], BF16, tag="vals")
                    nc.sync.dma_start(vals[:, :Lb], valc[:, o0:o1])
                    buf = spool.tile([128, bmax], F32, tag="buf")
                    nc.gpsimd.ap_gather(
                        buf[:, :Lb].rearrange("c (n d) -> c n d", d=1),
                        tbl[:].rearrange("c (n d) -> c n d", d=1),
                        gidx_s[:, o0 // 16:o1 // 16],
                        channels=128, num_elems=RP, d=1, num_idxs=Lb)
                    return buf, vals, Lb

                def batch_mult_scan(buf, vals, Lb):
                    for j0 in range(0, Lb, 512):
                        cj = min(512, Lb - j0)
                        vr = ppool.tile([128, 512], F32, tag="vr")
                        nc.tensor.matmul(vr[:, :cj], sel8s[:],
                                         vals[:, j0:j0 + cj],
                                         start=True, stop=True)
                        nc.vector.tensor_mul(buf[:, j0:j0 + cj],
                                             buf[:, j0:j0 + cj], vr[:, :cj])
                    nc.vector.tensor_tensor_scan(
                        buf[:, :Lb], ones_b(Lb), buf[:, :Lb], 0.0,
                        mybir.AluOpType.mult, mybir.AluOpType.add)

                def batch_extract(buf, Lb, q):
                    k0, k1 = q * BAT, min(SUBS, (q + 1) * BAT)
                    BE = (k1 - k0) * EXT
                    rxt = rxpool.tile([128, BAT * EXT], F32, tag="rxt")
                    nc.gpsimd.ap_gather(
                        rxt[:, :BE].rearrange("c (n d) -> c n d", d=1),
                        buf[:, :Lb].rearrange("c (n d) -> c n d", d=1),
                        eidx_s[:, k0 * EXT // 16:k1 * EXT // 16],
                        channels=128, num_elems=Lb, d=1, num_idxs=BE)
                    dd = ddpool.tile([128, BAT * EXT], F32, tag="dd")
                    nc.vector.tensor_sub(dd[:, 1:BE], rxt[:, 1:BE],
                                         rxt[:, :BE - 1])
                    return dd

                def ag2_trigger(c):
                    nc.gpsimd.collective_compute(
                        "AllGather", mybir.AluOpType.bypass,
                        ins=[agin2[c].opt()], outs=[agout2[c].opt()],
                        replica_groups=rg,
                    )

                # ---- layer 1: batched spmm -> h -> z2 -> agin2 (chunked AG2)
                cur = batch_gather(table, 0)
                trig = 0
                for q in range(NBAT):
                    buf, vals, Lb = cur
                    batch_mult_scan(buf, vals, Lb)
                    if q + 1 < NBAT:
                        cur = batch_gather(table, q + 1)
                    dd = batch_extract(buf, Lb, q)
                    k0, k1 = q * BAT, min(SUBS, (q + 1) * BAT)
                    for k in range(k0, min(k1, KSUB)):
                        b = k - k0
                        pb = ppool2.tile([16, SUBROWS], F32, tag="pb")
                        nc.tensor.matmul(
                            pb[:HID], sel1s[:],
                            dd[:, b * EXT + 1:b * EXT + 1 + SUBROWS],
                            start=True, stop=True,
                        )
                        hst = hpool.tile([HID, SUBROWS], F32, tag="hst")
                        nc.scalar.activation(
                            hst[:], pb[:HID],
                            mybir.ActivationFunctionType.Relu, bias=b1s[:])
                        if dbg:
                            nc.sync.dma_start(
                                dbg_h[:, k * SUBROWS:(k + 1) * SUBROWS], hst[:])
                        pz = ppool2.tile([OUT, SUBROWS], F32, tag="pz")
                        nc.tensor.matmul(pz[:], w2s[:], hst[:],
                                         start=True, stop=True)
                        stz = zpool.tile([16, SUBROWS], F32, tag="stz")
                        if k < 2:
                            nc.vector.memset(stz[:], 0.0)
                        nc.scalar.copy(stz[:OUT], pz[:])
                        c2 = k // SPC
                        nc.scalar.dma_start(
                            agin2[c2][:, (k % SPC) * SUBROWS:
                                      (k % SPC + 1) * SUBROWS],
                            stz[:],
                        )
                    while trig < NCH - 1 and SPC * (trig + 1) + 1 < k1:
                        ag2_trigger(trig)
                        trig += 1
                for c in range(trig, NCH):
                    ag2_trigger(c)

                # ---- table2 loads
                table2 = tabpool.tile([128, RP], F32, tag="table")
                for c in range(NCH):
                    nc.sync.dma_start(
                        table2[:, c * CH:(c + 1) * CH], agout2[c][:])
                if dbg:
                    nc.sync.dma_start(dbg_tab2[:], table2[:])

                # ---- layer 2: batched spmm -> transposed SEL -> zr row tiles
                zr = sfpool.tile([128, NRT2, OUT], F32, tag="zr")
                cur = batch_gather(table2, 0)
                for q in range(NBAT):
                    buf, vals, Lb = cur
                    batch_mult_scan(buf, vals, Lb)
                    if q + 1 < NBAT:
                        cur = batch_gather(table2, q + 1)
                    dd = batch_extract(buf, Lb, q)
                    k0, k1 = q * BAT, min(SUBS, (q + 1) * BAT)
                    for k in range(k0, min(k1, KSUB)):
                        b = k - k0
                        for h in range(NH):
                            t = k * NH + h
                            if t >= NRT:
                                break
                            pbT = ppool2.tile([128, 16], F32, tag="pbT")
                            nc.tensor.matmul(
                                pbT[:],
                                dd[:, b * EXT + 1 + 128 * h:
                                   b * EXT + 1 + 128 * h + 128],
                                sel2s[:],
                                start=True, stop=False,
                            )
                            nc.tensor.matmul(
                                pbT[:], onesr[:], b2rs[:],
                                start=False, stop=True,
                            )
                            nc.scalar.copy(zr[:, t, :], pbT[:, :OUT])

            if dbg:
                nc.sync.dma_start(
                    dbg_zr[:], zr[:].rearrange("p t j -> p (t j)"))
            # ---- log_softmax over OUT (batched)
            mx = sfpool.tile([128, NRT], F32, tag="mx")
            nc.vector.tensor_reduce(
                mx[:], zr[:, :NRT, :], axis=mybir.AxisListType.X,
                op=mybir.AluOpType.max)
            zs = sfpool.tile([128, OUT, NRT], F32, tag="zs")
            for j in range(OUT):
                nc.vector.tensor_sub(zs[:, j, :], zr[:, :NRT, j], mx[:])
            exb = sfpool.tile([128, OUT, NRT], F32, tag="exb")
            nc.scalar.activation(exb[:], zs[:], mybir.ActivationFunctionType.Exp)
            sm = sfpool.tile([128, NRT], F32, tag="sm")
            nc.vector.tensor_add(sm[:], exb[:, 0, :], exb[:, 1, :])
            for j in range(2, OUT):
                nc.vector.tensor_add(sm[:], sm[:], exb[:, j, :])
            lg = sfpool.tile([128, NRT], F32, tag="lg")
            nc.scalar.activation(lg[:], sm[:], mybir.ActivationFunctionType.Ln)
            for j in range(OUT):
                nc.vector.tensor_sub(zs[:, j, :], zs[:, j, :], lg[:])

            # ---- output: 7 transposes -> zo (NRT, 128, OUT) -> contiguous DMA
            zo = sfpool.tile([NRT, 128, OUT], F32, tag="zo")
            for j in range(OUT):
                pt = ppool2.tile([NRT, 128], F32, tag="pt")
                nc.tensor.matmul(
                    pt[:], zs[:, j, :], idents[:],
                    is_transpose=True, start=True, stop=True,
                )
                nc.scalar.copy(zo[:, :, j], pt[:])
            if NRT > 1:
                nc.sync.dma_start(
                    out[: (NRT - 1) * 128, :].rearrange(
                        "(t p) j -> t (p j)", p=128),
                    zo[: NRT - 1, :, :].rearrange("t p j -> t (p j)"),
                )
            nc.sync.dma_start(
                out[(NRT - 1) * 128:, :].rearrange("(t p) j -> t p j", t=1),
                zo[NRT - 1:NRT, :RLAST, :])

    nc.compile()
    return nc


def kernel(x, adj_row, adj_col, adj_val, W1, b1, W2, b2):
    from concourse import bass_utils

    cfg = FULL
    in_maps, glk, offs = prepare(
        np.asarray(x), np.asarray(adj_row), np.asarray(adj_col),
        np.asarray(adj_val), np.asarray(W1), np.asarray(b1),
        np.asarray(W2), np.asarray(b2), cfg,
    )
    nc = build(cfg, glk, offs)
    res = bass_utils.run_bass_kernel_spmd(nc, in_maps, core_ids=list(range(cfg.C)))
    outs = [res.results[m]["out"] for m in range(cfg.C)]
    return np.concatenate(outs, axis=0)[: cfg.N]



# revision 13
# speedup vs baseline: 1.1997x; 1.0310x over previous
"""GCN 2-layer kernel on 8 Trainium2 NeuronCores (Bass/Tile) — v2.

Sharding: core m owns dest rows [m*R, (m+1)*R). Edges partitioned by dest
row (core), then dest sub-chunk (SUBROWS rows, multiple of 128), then source
chunk (8 GPSIMD core-groups per NC), sorted by dest row.

Per layer SpMM (dest-sharded, AllGather of support rows):
  - table (128, R_pad) f32 in SBUF: partition 16g+f = feature f of source
    chunk g (= core g's rows), delivered by chunked partition-axis AllGather
  - ap_gather (GPSIMD): per-group edge-ordered gather from table
  - TensorE one-hot matmul broadcasts compact (8, L) edge vals to (128, L)
    PSUM (replaces the 16x-replicated HBM val stream of v1)
  - DVE: multiply, prefix scan; ap_gather #2 extracts prefix at per-row end
    positions; DVE shifted subtract -> per-(row,group) segment sums
  - TensorE one-hot SEL matmul sums the 8 group-partials
Layer 1 emits h in (feat, row) orientation and feeds z2 = W2^T h windows +
chunked AllGather#2 inline. Layer 2 uses the transposed SEL matmul
(dd stationary) to produce row-major tiles directly; log_softmax runs
batched, output leaves via 7 TensorE transposes + one contiguous DMA.

X@W1 runs in bf16 (X cast host-side; fp32 PSUM accumulation).
"""

import sys

for p in ("/opt/trn_rl_repo",):
    if p not in sys.path:
        sys.path.insert(0, p)

import numpy as np
import ml_dtypes

import concourse.bass as bass
import concourse.mybir as mybir
import concourse.tile as tile
from concourse import bacc, library_config

F32 = mybir.dt.float32
F32R = mybir.dt.float32r
BF16 = mybir.dt.bfloat16
I16 = mybir.dt.int16
NPBF = ml_dtypes.bfloat16


class Cfg:
    def __init__(self, N, E, IN, HID, OUT, SUBS, SUBROWS, NW, NCH, BAT):
        self.N = N
        self.E = E
        self.IN = IN
        self.HID = HID          # 16
        self.OUT = OUT          # 7
        self.C = 8
        self.R = N // 8         # real rows per core / source chunk
        self.SUBS = SUBS
        self.SUBROWS = SUBROWS  # rows per sub-chunk; multiple of 128
        self.RP = SUBS * SUBROWS  # padded rows per core
        self.NH = SUBROWS // 128  # 128-row tiles per sub
        self.NW = NW            # phase-A column windows
        self.WCOL = self.RP // NW
        self.NCH = NCH          # allgather chunks
        self.BAT = BAT          # subs per batched gather/scan/extract
        self.CH = self.RP // NCH
        self.EXT = -(-(SUBROWS + 1) // 32) * 32  # //16 even: 4B-aligned idx slices
        self.KC = -(-IN // 128)
        self.KLAST = IN - (self.KC - 1) * 128
        self.NRT2 = self.RP // 128       # row tiles incl. padding
        self.NRT = -(-self.R // 128)     # row tiles with real rows
        assert SUBROWS % 128 == 0
        assert self.RP >= self.R and self.RP % NW == 0 and self.WCOL <= 512
        assert self.CH % self.WCOL == 0 and self.CH % SUBROWS == 0
        assert self.RP * 4 // 4 <= 2**15  # ap_gather num_elems limit (f32)


FULL = Cfg(N=100_000, E=3_200_000, IN=1433, HID=16, OUT=7,
           SUBS=25, SUBROWS=512, NW=25, NCH=1, BAT=4)


def prepare(x, adj_row, adj_col, adj_val, W1, b1, W2, b2, cfg):
    """Host preprocessing: build per-core input maps (vectorized numpy)."""
    N, E, R, C = cfg.N, cfg.E, cfg.R, cfg.C
    SUBS, SUBROWS, EXT = cfg.SUBS, cfg.SUBROWS, cfg.EXT

    adj_row = np.asarray(adj_row)
    adj_col = np.asarray(adj_col)
    adj_val = np.asarray(adj_val, dtype=np.float32)

    core = adj_row // R
    row_rel = adj_row - core * R
    sub = row_rel // SUBROWS
    grp = adj_col // R
    order = np.lexsort((adj_row, grp, sub, core))
    r_s = adj_row[order]
    c_s = adj_col[order]
    v_s = adj_val[order]
    core_s = core[order]
    grp_s = grp[order]
    sub_s = sub[order]

    key = (core_s * SUBS + sub_s) * C + grp_s
    ncell = C * SUBS * C
    starts = np.searchsorted(key, np.arange(ncell))
    ends = np.searchsorted(key, np.arange(ncell) + 1)
    cnt = (ends - starts).reshape(C, SUBS, C)

    glk = cnt.max(axis=(0, 2)) + 1
    # round to 32 so offs//16 stays even (4B-aligned int16 idx slices for
    # the gather ucode, which reads the index stream as uint32)
    glk = (-(-glk // 32) * 32).astype(np.int64)
    glk = np.maximum(glk, 32)
    gtot = int(glk.sum())
    offs = np.concatenate([[0], np.cumsum(glk)])

    # vectorized stream fill: slot = offs[sub] + 1 + rank-within-cell
    rank = np.arange(E) - starts[key]
    pos = offs[sub_s] + 1 + rank
    gidx_full = np.zeros((C, C, gtot), np.int16)
    valc_full = np.zeros((C, C, gtot), np.float32)
    gidx_full[core_s, grp_s, pos] = (c_s - grp_s * R).astype(np.int16)
    valc_full[core_s, grp_s, pos] = v_s

    # extraction positions: per (core, sub, grp, row) right-searchsorted
    rr = (r_s - core_s * R) - sub_s * SUBROWS
    idx4 = key * SUBROWS + rr
    cnt4 = np.bincount(idx4, minlength=ncell * SUBROWS).reshape(
        C, SUBS, C, SUBROWS)
    pos4 = cnt4.cumsum(axis=3)
    ex = np.zeros((C, SUBS, C, EXT), np.int64)
    ex[..., 1:1 + SUBROWS] = pos4
    ex[..., 1 + SUBROWS:] = pos4[..., -1:]
    # batch-local stream offsets: sub k's extract indices point into the
    # batched gather buffer [offs[k0], offs[k1])
    for k in range(SUBS):
        k0 = (k // cfg.BAT) * cfg.BAT
        ex[:, k] += offs[k] - offs[k0]
    assert ex.max() < 2**15
    ex = ex.astype(np.int16)

    def wrap16(a):
        # (8, L) -> (128, L//16): out[16g+p, s] = a[g, s*16+p]
        Cg, L = a.shape
        return np.ascontiguousarray(
            a.reshape(Cg, L // 16, 16).transpose(0, 2, 1).reshape(
                Cg * 16, L // 16))

    W1f = np.asarray(W1, dtype=np.float32)
    w1p = np.zeros((cfg.KC * 128, cfg.HID), np.float32)
    w1p[: cfg.IN] = W1f
    w1p = w1p.astype(NPBF)

    sel1 = np.zeros((128, 16), np.float32)
    sel1[np.arange(128), np.arange(128) % 16] = 1.0
    sel2 = np.zeros((128, 16), np.float32)
    for p in range(128):
        if p % 16 < cfg.OUT:
            sel2[p, p % 16] = 1.0
    b2r = np.zeros((1, 16), np.float32)
    b2r[0, : cfg.OUT] = np.asarray(b2, dtype=np.float32).reshape(-1)
    sel8 = np.zeros((8, 128), np.float32)
    for g in range(8):
        sel8[g, 16 * g:16 * g + 16] = 1.0
    sel8 = sel8.astype(NPBF)
    ident = np.eye(128, dtype=np.float32)

    x = np.asarray(x)
    in_maps = []
    for m in range(C):
        xt = np.ascontiguousarray(
            x[m * R:(m + 1) * R].T.astype(NPBF))
        e2 = np.ascontiguousarray(
            ex[m].transpose(1, 0, 2).reshape(C, SUBS * EXT))
        in_maps.append(
            dict(
                xt=xt,
                w1p=w1p,
                b1=np.ascontiguousarray(
                    np.asarray(b1, np.float32).reshape(cfg.HID, 1)),
                w2=np.ascontiguousarray(np.asarray(W2, np.float32)),
                gidx=wrap16(gidx_full[m]),
                eidx=wrap16(e2),
                valc=np.ascontiguousarray(valc_full[m].astype(NPBF)),
                sel1=sel1,
                sel2=sel2,
                b2r=b2r,
                sel8=sel8,
                ident=ident,
            )
        )
    return in_maps, glk, offs


def build(cfg, glk, offs, dbg=False):
    nc = bacc.Bacc("TRN2", target_bir_lowering=False, debug=False,
                   num_devices=cfg.C)
    R, RP, HID, OUT = cfg.R, cfg.RP, cfg.HID, cfg.OUT
    SUBS, SUBROWS, EXT, NH = cfg.SUBS, cfg.SUBROWS, cfg.EXT, cfg.NH
    NW, WCOL, NCH, CH = cfg.NW, cfg.WCOL, cfg.NCH, cfg.CH
    KC, KLAST = cfg.KC, cfg.KLAST
    NRT2, NRT = cfg.NRT2, cfg.NRT
    gtot = int(glk.sum())
    glkmax = int(max(glk))
    WPC = NW // NCH    # windows per AG1 chunk
    SPC = SUBS // NCH  # subs per AG2 chunk
    # last sub with any real rows
    KSUB = -(-R // SUBROWS)  # number of subs covering real rows
    RLAST = R - (NRT - 1) * 128  # rows in last real 128-tile

    xt = nc.dram_tensor("xt", [cfg.IN, R], BF16, kind="ExternalInput").ap()
    w1p = nc.dram_tensor("w1p", [KC * 128, HID], BF16, kind="ExternalInput").ap()
    b1 = nc.dram_tensor("b1", [HID, 1], F32, kind="ExternalInput").ap()
    w2 = nc.dram_tensor("w2", [HID, OUT], F32, kind="ExternalInput").ap()
    gidx = nc.dram_tensor("gidx", [128, gtot // 16], I16, kind="ExternalInput").ap()
    eidx = nc.dram_tensor("eidx", [128, SUBS * EXT // 16], I16, kind="ExternalInput").ap()
    valc = nc.dram_tensor("valc", [8, gtot], BF16, kind="ExternalInput").ap()
    sel1 = nc.dram_tensor("sel1", [128, 16], F32, kind="ExternalInput").ap()
    sel2 = nc.dram_tensor("sel2", [128, 16], F32, kind="ExternalInput").ap()
    b2r = nc.dram_tensor("b2r", [1, 16], F32, kind="ExternalInput").ap()
    sel8 = nc.dram_tensor("sel8", [8, 128], BF16, kind="ExternalInput").ap()
    ident = nc.dram_tensor("ident", [128, 128], F32, kind="ExternalInput").ap()
    out = nc.dram_tensor("out", [R, OUT], F32, kind="ExternalOutput").ap()
    if dbg:
        dbg_tab = nc.dram_tensor("dbg_tab", [128, RP], F32, kind="ExternalOutput").ap()
        dbg_tab2 = nc.dram_tensor("dbg_tab2", [128, RP], F32, kind="ExternalOutput").ap()
        dbg_h = nc.dram_tensor("dbg_h", [HID, RP], F32, kind="ExternalOutput").ap()
        dbg_zr = nc.dram_tensor("dbg_zr", [128, NRT2 * OUT], F32, kind="ExternalOutput").ap()

    rg = [list(range(cfg.C))]

    with tile.TileContext(nc) as tc:
        BAT = cfg.BAT
        NBAT = -(-SUBS // BAT)
        bmax = 0
        for q in range(NBAT):
            k0, k1 = q * BAT, min(SUBS, (q + 1) * BAT)
            bmax = max(bmax, int(offs[k1] - offs[k0]))
        with (
            tc.tile_pool(name="const", bufs=1) as cpool,
            tc.tile_pool(name="tab", bufs=1) as tabpool,
            tc.tile_pool(name="soft", bufs=1) as sfpool,
            tc.tile_pool(name="hz", bufs=2) as hpool,
            tc.tile_pool(name="stz", bufs=2) as zpool,
            tc.tile_pool(name="st1", bufs=2) as stpool,
            tc.tile_pool(name="psA", bufs=2, space="PSUM") as ppool,
            tc.tile_pool(name="psB", bufs=1, space="PSUM") as ppool2,
            tc.tile_pool(name="dram", bufs=1, space="DRAM") as dpool,
        ):
            nc.gpsimd.load_library(library_config.ap_gather)

            # ---- consts + index prefetch (sync queue)
            w1s = cpool.tile([128, KC, HID], BF16)
            nc.sync.dma_start(w1s[:], w1p.rearrange("(k p) m -> p k m", p=128))
            b1s = cpool.tile([HID, 1], F32)
            nc.sync.dma_start(b1s[:], b1[:])
            w2s = cpool.tile([HID, OUT], F32)
            nc.sync.dma_start(w2s[:], w2[:])
            sel1s = cpool.tile([128, 16], F32)
            nc.sync.dma_start(sel1s[:], sel1[:])
            sel2s = cpool.tile([128, 16], F32)
            nc.sync.dma_start(sel2s[:], sel2[:])
            b2rs = cpool.tile([1, 16], F32)
            nc.sync.dma_start(b2rs[:], b2r[:])
            sel8s = cpool.tile([8, 128], BF16)
            nc.sync.dma_start(sel8s[:], sel8[:])
            idents = cpool.tile([128, 128], F32)
            nc.sync.dma_start(idents[:], ident[:])
            gidx_s = cpool.tile([128, gtot // 16], I16)
            nc.sync.dma_start(gidx_s[:], gidx[:])
            eidx_s = cpool.tile([128, SUBS * EXT // 16], I16)
            nc.sync.dma_start(eidx_s[:], eidx[:])
            ones1 = cpool.tile([128, 1], F32)
            nc.vector.memset(ones1[:], 1.0)
            onesr = cpool.tile([1, 128], F32)
            nc.vector.memset(onesr[:], 1.0)

            def ones_b(L):
                # stride-0 broadcast of the per-partition 1.0 along free dim
                return bass.AP(ones1.tensor, ones1.offset,
                               [ones1.ap[0], [0, L]])

            # ---- DRAM collective chunk tiles
            agin1 = [dpool.tile([HID, CH], F32, name=f"agin1_{c}")
                     for c in range(NCH)]
            agout1 = [dpool.tile([128, CH], F32, addr_space="Shared",
                                 name=f"agout1_{c}") for c in range(NCH)]
            agin2 = [dpool.tile([16, CH], F32, name=f"agin2_{c}")
                     for c in range(NCH)]
            agout2 = [dpool.tile([128, CH], F32, addr_space="Shared",
                                 name=f"agout2_{c}") for c in range(NCH)]

            # ---- phase A: (X @ W1)^T windows -> agin1 chunks, chunked AG1
            # (xw pool scoped so its SBUF returns before the stream buffers)
            with tc.tile_pool(name="xw", bufs=2) as xpool:
                for w in range(NW):
                    c0 = w * WCOL
                    cols = min(WCOL, R - c0) if c0 < R else 0
                    ch = w // WPC
                    if cols > 0:
                        xw = xpool.tile([128, KC, WCOL], BF16, tag="xw")
                        if KC > 1:
                            nc.sync.dma_start(
                                xw[:, : KC - 1, :cols],
                                xt[: (KC - 1) * 128, c0:c0 + cols].rearrange(
                                    "(k p) c -> p k c", p=128),
                            )
                        nc.sync.dma_start(
                            xw[:KLAST, KC - 1, :cols],
                            xt[(KC - 1) * 128:, c0:c0 + cols],
                        )
                        pa = ppool.tile([HID, WCOL], F32, tag="pa")
                        for k in range(KC):
                            kp = 128 if k < KC - 1 else KLAST
                            nc.tensor.matmul(
                                pa[:, :cols], w1s[:kp, k, :], xw[:kp, k, :cols],
                                start=(k == 0), stop=(k == KC - 1),
                            )
                        st1 = stpool.tile([HID, WCOL], F32, tag="st1")
                        nc.scalar.copy(st1[:, :cols], pa[:, :cols])
                        nc.scalar.dma_start(
                            agin1[ch][:, (w % WPC) * WCOL:(w % WPC) * WCOL + cols],
                            st1[:, :cols],
                        )
                    if w % WPC == WPC - 1:
                        nc.gpsimd.collective_compute(
                            "AllGather", mybir.AluOpType.bypass,
                            ins=[agin1[ch].opt()], outs=[agout1[ch].opt()],
                            replica_groups=rg,
                        )

            # ---- table1 loads (sync queue, after phase A)
            table = tabpool.tile([128, RP], F32, tag="table")
            for c in range(NCH):
                nc.sync.dma_start(table[:, c * CH:(c + 1) * CH], agout1[c][:])
            if dbg:
                nc.sync.dma_start(dbg_tab[:], table[:])

            with (
                tc.tile_pool(name="stream", bufs=2) as spool,
                tc.tile_pool(name="vals", bufs=1) as vpool,
                tc.tile_pool(name="rx", bufs=1) as rxpool,
                tc.tile_pool(name="ddp", bufs=2) as ddpool,
            ):
                def batch_gather(tbl, q):
                    k0, k1 = q * BAT, min(SUBS, (q + 1) * BAT)
                    o0, o1 = int(offs[k0]), int(offs[k1])
                    Lb = o1 - o0
                    vals = vpool.tile([8, bmax], BF16, tag="vals")
                    nc.sync.dma_start(vals[:, :Lb], valc[:, o0:o1])
                    buf = spool.tile([128, bmax], F32, tag="buf")
                    nc.gpsimd.ap_gather(
                        buf[:, :Lb].rearrange("c (n d) -> c n d", d=1),
                        tbl[:].rearrange("c (n d) -> c n d", d=1),
                        gidx_s[:, o0 // 16:o1 // 16],
                        channels=128, num_elems=RP, d=1, num_idxs=Lb)
                    return buf, vals, Lb

                def batch_mult_scan(buf, vals, Lb):
                    for j0 in range(0, Lb, 512):
                        cj = min(512, Lb - j0)
                        vr = ppool.tile([128, 512], F32, tag="vr")
                        nc.tensor.matmul(vr[:, :cj], sel8s[:],
                                         vals[:, j0:j0 + cj],
                                         start=True, stop=True)
                        nc.vector.tensor_mul(buf[:, j0:j0 + cj],
                                             buf[:, j0:j0 + cj], vr[:, :cj])
                    nc.vector.tensor_tensor_scan(
                        buf[:, :Lb], ones_b(Lb), buf[:, :Lb], 0.0,
                        mybir.AluOpType.mult, mybir.AluOpType.add)

                def batch_extract(buf, Lb, q):
                    k0, k1 = q * BAT, min(SUBS, (q + 1) * BAT)
                    BE = (k1 - k0) * EXT
                    rxt = rxpool.tile([128, BAT * EXT], F32, tag="rxt")
                    nc.gpsimd.ap_gather(
                        rxt[:, :BE].rearrange("c (n d) -> c n d", d=1),
                        buf[:, :Lb].rearrange("c (n d) -> c n d", d=1),
                        eidx_s[:, k0 * EXT // 16:k1 * EXT // 16],
                        channels=128, num_elems=Lb, d=1, num_idxs=BE)
                    dd = ddpool.tile([128, BAT * EXT], F32, tag="dd")
                    nc.vector.tensor_sub(dd[:, 1:BE], rxt[:, 1:BE],
                                         rxt[:, :BE - 1])
                    return dd

                def ag2_trigger(c):
                    nc.gpsimd.collective_compute(
                        "AllGather", mybir.AluOpType.bypass,
                        ins=[agin2[c].opt()], outs=[agout2[c].opt()],
                        replica_groups=rg,
                    )

                # ---- layer 1: batched spmm -> h -> z2 -> agin2 (chunked AG2)
                cur = batch_gather(table, 0)
                trig = 0
                for q in range(NBAT):
                    buf, vals, Lb = cur
                    batch_mult_scan(buf, vals, Lb)
                    if q + 1 < NBAT:
                        cur = batch_gather(table, q + 1)
                    dd = batch_extract(buf, Lb, q)
                    k0, k1 = q * BAT, min(SUBS, (q + 1) * BAT)
                    for k in range(k0, min(k1, KSUB)):
                        b = k - k0
                        pb = ppool2.tile([16, SUBROWS], F32, tag="pb")
                        nc.tensor.matmul(
                            pb[:HID], sel1s[:],
                            dd[:, b * EXT + 1:b * EXT + 1 + SUBROWS],
                            start=True, stop=True,
                        )
                        hst = hpool.tile([HID, SUBROWS], F32, tag="hst")
                        nc.scalar.activation(
                            hst[:], pb[:HID],
                            mybir.ActivationFunctionType.Relu, bias=b1s[:])
                        if dbg:
                            nc.sync.dma_start(
                                dbg_h[:, k * SUBROWS:(k + 1) * SUBROWS], hst[:])
                        pz = ppool2.tile([OUT, SUBROWS], F32, tag="pz")
                        nc.tensor.matmul(pz[:], w2s[:], hst[:],
                                         start=True, stop=True)
                        stz = zpool.tile([16, SUBROWS], F32, tag="stz")
                        if k < 2:
                            nc.vector.memset(stz[:], 0.0)
                        nc.scalar.copy(stz[:OUT], pz[:])
                        c2 = k // SPC
                        nc.scalar.dma_start(
                            agin2[c2][:, (k % SPC) * SUBROWS:
                                      (k % SPC + 1) * SUBROWS],
                            stz[:],
                        )
                    while trig < NCH - 1 and SPC * (trig + 1) + 1 < k1:
                        ag2_trigger(trig)
                        trig += 1
                for c in range(trig, NCH):
                    ag2_trigger(c)

                # ---- table2 loads
                table2 = tabpool.tile([128, RP], F32, tag="table")
                for c in range(NCH):
                    nc.sync.dma_start(
                        table2[:, c * CH:(c + 1) * CH], agout2[c][:])
                if dbg:
                    nc.sync.dma_start(dbg_tab2[:], table2[:])

                # ---- layer 2: batched spmm -> transposed SEL -> zr row tiles
                zr = sfpool.tile([128, NRT2, OUT], F32, tag="zr")
                cur = batch_gather(table2, 0)
                for q in range(NBAT):
                    buf, vals, Lb = cur
                    batch_mult_scan(buf, vals, Lb)
                    if q + 1 < NBAT:
                        cur = batch_gather(table2, q + 1)
                    dd = batch_extract(buf, Lb, q)
                    k0, k1 = q * BAT, min(SUBS, (q + 1) * BAT)
                    for k in range(k0, min(k1, KSUB)):
                        b = k - k0
                        for h in range(NH):
                            t = k * NH + h
                            if t >= NRT:
                                break
                            pbT = ppool2.tile([128, 16], F32, tag="pbT")
                            nc.tensor.matmul(
                                pbT[:],
                                dd[:, b * EXT + 1 + 128 * h:
                                   b * EXT + 1 + 128 * h + 128],
                                sel2s[:],
                                start=True, stop=False,
                            )
                            nc.tensor.matmul(
                                pbT[:], onesr[:], b2rs[:],
                                start=False, stop=True,
                            )
                            nc.scalar.copy(zr[:, t, :], pbT[:, :OUT])

            if dbg:
                nc.sync.dma_start(
                    dbg_zr[:], zr[:].rearrange("p t j -> p (t j)"))
            # ---- log_softmax over OUT (batched)
            mx = sfpool.tile([128, NRT], F32, tag="mx")
            nc.vector.tensor_reduce(
                mx[:], zr[:, :NRT, :], axis=mybir.AxisListType.X,
                op=mybir.AluOpType.max)
            zs = sfpool.tile([128, OUT, NRT], F32, tag="zs")
            for j in range(OUT):
                nc.vector.tensor_sub(zs[:, j, :], zr[:, :NRT, j], mx[:])
            exb = sfpool.tile([128, OUT, NRT], F32, tag="exb")
            nc.scalar.activation(exb[:], zs[:], mybir.ActivationFunctionType.Exp)
            sm = sfpool.tile([128, NRT], F32, tag="sm")
            nc.vector.tensor_add(sm[:], exb[:, 0, :], exb[:, 1, :])
            for j in range(2, OUT):
                nc.vector.tensor_add(sm[:], sm[:], exb[:, j, :])
            lg = sfpool.tile([128, NRT], F32, tag="lg")
            nc.scalar.activation(lg[:], sm[:], mybir.ActivationFunctionType.Ln)
            for j in range(OUT):
                nc.vector.tensor_sub(zs[:, j, :], zs[:, j, :], lg[:])

            # ---- output: 7 transposes -> zo (NRT, 128, OUT) -> contiguous DMA
            zo = sfpool.tile([NRT, 128, OUT], F32, tag="zo")
            for j in range(OUT):
                pt = ppool2.tile([NRT, 128], F32, tag="pt")
                nc.tensor.matmul(
                    pt[:], zs[:, j, :], idents[:],
                    is_transpose=True, start=True, stop=True,
                )
                nc.scalar.copy(zo[:, :, j], pt[:])
            if NRT > 1:
                nc.sync.dma_start(
                    out[: (NRT - 1) * 128, :].rearrange(
                        "(t p) j -> t (p j)", p=128),
                    zo[: NRT - 1, :, :].rearrange("t p j -> t (p j)"),
                )
            nc.sync.dma_start(
                out[(NRT - 1) * 128:, :].rearrange("(t p) j -> t p j", t=1),
                zo[NRT - 1:NRT, :RLAST, :])

    nc.compile()
    return nc


def kernel(x, adj_row, adj_col, adj_val, W1, b1, W2, b2):
    from concourse import bass_utils

    cfg = FULL
    in_maps, glk, offs = prepare(
        np.asarray(x), np.asarray(adj_row), np.asarray(adj_col),
        np.asarray(adj_val), np.asarray(W1), np.asarray(b1),
        np.asarray(W2), np.asarray(b2), cfg,
    )
    nc = build(cfg, glk, offs)
    res = bass_utils.run_bass_kernel_spmd(nc, in_maps, core_ids=list(range(cfg.C)))
    outs = [res.results[m]["out"] for m in range(cfg.C)]
    return np.concatenate(outs, axis=0)[: cfg.N]

